# revision 1
# baseline (speedup 1.0000x reference)
"""GAT DirSeq (conv_in + conv_out on flipped edges) Trainium2 kernel.

Strategy (edge partition by destination block, per sharding hint):
  - Nodes are grouped into 128-node blocks; blocks are sharded over 8 cores.
  - Node phase (replicated on every core): T_in = [h_in | a_src_in],
    T_out = [h_out | a_src_out] (fp16, DRAM tables), A = [a_dst_in | a_dst_out].
    h = x @ W computed by TensorE; the per-head alpha reductions are folded
    into the same matmul as extra columns (V = W_head @ a_head).
  - Edge phase: per destination block, indirect-DMA gather of source rows,
    softmax expressed as unnormalized weighted sum (exact same math as the
    reference since the max-subtraction cancels): num = sum ex*h, den = sum ex,
    scattered into PSUM via one-hot matmuls; out = num/den + bias.
"""

import math
from contextlib import ExitStack

import numpy as np

N = 100000
E = 800000
_LAST = {}
D_IN = 128
HEADS = 8
C = 16
NEG_SLOPE = 0.2
P = 128


# ---------------------------------------------------------------- host prep
def _edge_arrays(key, oth, NB, SENT):
    """Per-direction edge layout: for each 128-node block of `key`, the list of
    edges targeting it, padded to S*128, laid out so edge at (partition p,
    slot s) is sorted position s*128+p within the block."""
    order = np.argsort(key, kind="stable")
    k_s = key[order].astype(np.int64)
    o_s = oth[order].astype(np.int64)
    blk = k_s // P
    cnt = np.bincount(blk, minlength=NB)
    S = max(1, int(math.ceil(cnt.max() / P)))
    starts = np.zeros(NB + 1, np.int64)
    np.cumsum(cnt, out=starts[1:])
    pos = np.arange(k_s.size, dtype=np.int64) - starts[blk]
    p = pos % P
    s = pos // P
    gidx = np.full((NB, P, S), SENT, np.int32)
    gdst = np.full((NB, P, S), SENT, np.int32)
    ldst = np.zeros((NB, P, S), np.float16)
    gidx[blk, p, s] = o_s
    gdst[blk, p, s] = k_s
    ldst[blk, p, s] = (k_s % P).astype(np.float16)
    return gidx, gdst, ldst, S


def _build_and_run(x, ei, W_in, a_src_in, a_dst_in, b_in, W_out, a_src_out,
                   a_dst_out, b_out, n_nodes, n_edges, n_cores=8):
    import concourse.bacc as bacc
    import concourse.bass as bass
    import concourse.mybir as mybir
    import concourse.tile as tile
    from concourse.bass_utils import run_bass_kernel_spmd

    fp16 = mybir.dt.float16
    f32 = mybir.dt.float32
    i32 = mybir.dt.int32

    NBLK_C = int(math.ceil(n_nodes / P / n_cores))  # blocks per core
    NB = NBLK_C * n_cores
    NPAD = NB * P
    SENT = NPAD  # sentinel row id
    NR = NPAD + 1

    src, dst = ei[0].astype(np.int64), ei[1].astype(np.int64)
    gi_i, gd_i, ld_i, S_IN = _edge_arrays(dst, src, NB, SENT)
    gi_o, gd_o, ld_o, S_OUT = _edge_arrays(src, dst, NB, SENT)
    SMAX = max(S_IN, S_OUT)
    KI = 2 * (S_IN + S_OUT)
    KL = S_IN + S_OUT

    # folded parameter matrix [D_IN, 288]
    Vsrc_in = np.stack([W_in[:, h * C:(h + 1) * C] @ a_src_in[h] for h in range(HEADS)], 1)
    Vdst_in = np.stack([W_in[:, h * C:(h + 1) * C] @ a_dst_in[h] for h in range(HEADS)], 1)
    Vsrc_out = np.stack([W_out[:, h * C:(h + 1) * C] @ a_src_out[h] for h in range(HEADS)], 1)
    Vdst_out = np.stack([W_out[:, h * C:(h + 1) * C] @ a_dst_out[h] for h in range(HEADS)], 1)
    wcat = np.concatenate(
        [W_in, Vsrc_in, W_out, Vsrc_out, Vdst_in, Vdst_out], axis=1
    ).astype(np.float16)  # [128, 288]

    xT = np.zeros((D_IN, NPAD), np.float16)
    xT[:, :n_nodes] = x.T.astype(np.float16)

    iota = np.tile(np.arange(P, dtype=np.float16), (P, SMAX)).reshape(P, SMAX * P)
    sent = np.zeros((1, 288), np.float16)
    sent[0, 128:136] = -1e4   # T_in alpha_src
    sent[0, 264:272] = -1e4   # T_out alpha_src
    bias = np.tile((b_in + b_out).astype(np.float32)[None, :], (P, 1))

    # per-core edge info, SBUF layout [128, NBLK_C * K]
    einfo, ldsta = [], []
    for k in range(n_cores):
        sl = slice(k * NBLK_C, (k + 1) * NBLK_C)
        e = np.concatenate([gi_i[sl], gd_i[sl], gi_o[sl], gd_o[sl]], axis=2)
        einfo.append(np.ascontiguousarray(e.transpose(1, 0, 2).reshape(P, NBLK_C * KI)))
        l = np.concatenate([ld_i[sl], ld_o[sl]], axis=2)
        ldsta.append(np.ascontiguousarray(l.transpose(1, 0, 2).reshape(P, NBLK_C * KL)))

    # ------------------------------------------------------------- program
    nc = bacc.Bacc(None, target_bir_lowering=False, debug=False)
    ctx = ExitStack()

    p_xT = nc.declare_dram_parameter("xT", [P, NPAD], fp16, isOutput=False)
    p_wcat = nc.declare_dram_parameter("wcat", [P, 288], fp16, isOutput=False)
    p_iota = nc.declare_dram_parameter("iota", [P, SMAX * P], fp16, isOutput=False)
    p_sent = nc.declare_dram_parameter("sent", [1, 288], fp16, isOutput=False)
    p_bias = nc.declare_dram_parameter("bias", [P, 128], f32, isOutput=False)
    p_einfo = nc.declare_dram_parameter("einfo", [P, NBLK_C * KI], i32, isOutput=False)
    p_ldst = nc.declare_dram_parameter("ldst", [P, NBLK_C * KL], fp16, isOutput=False)
    p_out = nc.declare_dram_parameter("out", [NBLK_C * P, 128], f32, isOutput=True)

    T_in = nc.dram_tensor("T_in", [NR, 136], fp16)
    T_out = nc.dram_tensor("T_out", [NR, 136], fp16)
    A = nc.dram_tensor("A", [NR, 16], fp16)

    NBAT = 16

    with tile.TileContext(nc) as tc:
        with (
            tc.tile_pool(name="const", bufs=1) as cpool,
            tc.tile_pool(name="xc", bufs=2) as xpool,
            tc.tile_pool(name="nstage", bufs=2) as spool,
            tc.tile_pool(name="npsum", bufs=2, space="PSUM") as npsum,
            tc.tile_pool(name="gath", bufs=3) as gpool,
            tc.tile_pool(name="agath", bufs=3) as apool,
            tc.tile_pool(name="oneh", bufs=3) as opool,
            tc.tile_pool(name="small", bufs=4) as mpool,
            tc.tile_pool(name="epsum", bufs=2, space="PSUM") as epsum,
            tc.tile_pool(name="epi", bufs=3) as dpool,
        ):
            wcat_s = cpool.tile([P, 288], fp16)
            nc.sync.dma_start(out=wcat_s[:], in_=p_wcat[:])
            iota_s = cpool.tile([P, SMAX * P], fp16)
            nc.sync.dma_start(out=iota_s[:], in_=p_iota[:])
            bias_s = cpool.tile([P, 128], f32)
            nc.sync.dma_start(out=bias_s[:], in_=p_bias[:])
            einfo_s = cpool.tile([P, NBLK_C * KI], i32)
            nc.sync.dma_start(out=einfo_s[:], in_=p_einfo[:])
            ldst_s = cpool.tile([P, NBLK_C * KL], fp16)
            nc.sync.dma_start(out=ldst_s[:], in_=p_ldst[:])

            # sentinel rows
            nc.sync.dma_start(out=T_in[NPAD:NR, :], in_=p_sent[:, 0:136])
            nc.sync.dma_start(out=T_out[NPAD:NR, :], in_=p_sent[:, 136:272])
            nc.sync.dma_start(out=A[NPAD:NR, :], in_=p_sent[:, 272:288])

            # ---------------- node phase ----------------
            for g0 in range(0, NB, NBAT):
                nb = min(NBAT, NB - g0)
                xc = xpool.tile([P, nb * P], fp16, tag="xc")
                nc.sync.dma_start(out=xc[:], in_=p_xT[:, g0 * P:(g0 + nb) * P])
                stage = spool.tile([P, nb * 288], fp16, tag="nstage")
                for j in range(nb):
                    ps = npsum.tile([P, 288], f32, tag="nps")
                    nc.tensor.matmul(out=ps[:], lhsT=xc[:, j * P:(j + 1) * P],
                                     rhs=wcat_s[:], start=True, stop=True)
                    dstg = stage[:, j * 288:(j + 1) * 288]
                    if j % 2 == 0:
                        nc.scalar.copy(out=dstg, in_=ps[:])
                    else:
                        nc.vector.tensor_copy(out=dstg, in_=ps[:])
                st3 = stage[:].rearrange("p (j c) -> p j c", c=288)
                r0 = g0 * P
                rows = nb * P
                tin_v = T_in[r0:r0 + rows, :].rearrange("(j p) c -> p j c", p=P)
                nc.sync.dma_start(out=tin_v, in_=st3[:, :, 0:136])
                tout_v = T_out[r0:r0 + rows, :].rearrange("(j p) c -> p j c", p=P)
                nc.sync.dma_start(out=tout_v, in_=st3[:, :, 136:272])
                a_v = A[r0:r0 + rows, :].rearrange("(j p) c -> p j c", p=P)
                nc.sync.dma_start(out=a_v, in_=st3[:, :, 272:288])

            # ---------------- edge phase ----------------
            for b in range(NBLK_C):
                outs_d = []
                for d in range(2):
                    S = S_IN if d == 0 else S_OUT
                    T = T_in if d == 0 else T_out
                    eoff = b * KI + (0 if d == 0 else 2 * S_IN)
                    loff = b * KL + (0 if d == 0 else S_IN)

                    gath = gpool.tile([P, S * 136], fp16, tag="gath")
                    for s in range(S):
                        nc.gpsimd.indirect_dma_start(
                            out=gath[:, s * 136:(s + 1) * 136], out_offset=None,
                            in_=T[:, :],
                            in_offset=bass.IndirectOffsetOnAxis(
                                ap=einfo_s[:, eoff + s:eoff + s + 1], axis=0))
                    agath = apool.tile([P, S * 16], fp16, tag="agath")
                    for s in range(S):
                        nc.gpsimd.indirect_dma_start(
                            out=agath[:, s * 16:(s + 1) * 16], out_offset=None,
                            in_=A[:, :],
                            in_offset=bass.IndirectOffsetOnAxis(
                                ap=einfo_s[:, eoff + S + s:eoff + S + s + 1],
                                axis=0))

                    oneh = opool.tile([P, S * P], fp16, tag="oneh")
                    ld_b = ldst_s[:, loff:loff + S].rearrange("p (s o) -> p s o", o=1)
                    nc.vector.tensor_tensor(
                        out=oneh[:].rearrange("p (s e) -> p s e", e=P),
                        in0=ld_b.to_broadcast([P, S, P]),
                        in1=iota_s[:, 0:S * P].rearrange("p (s e) -> p s e", e=P),
                        op=mybir.AluOpType.is_equal)

                    g3 = gath[:].rearrange("p (s c) -> p s c", c=136)
                    a3 = agath[:].rearrange("p (s c) -> p s c", c=16)
                    aex = mpool.tile([P, S * 8], fp16, tag="aex")
                    nc.vector.tensor_tensor(
                        out=aex[:].rearrange("p (s h) -> p s h", h=8),
                        in0=g3[:, :, 128:136], in1=a3[:, :, d * 8:d * 8 + 8],
                        op=mybir.AluOpType.add)
                    lrl0 = mpool.tile([P, S * 8], fp16, tag="lrl0")
                    nc.vector.tensor_scalar(out=lrl0[:], in0=aex[:],
                                            scalar1=NEG_SLOPE, scalar2=None,
                                            op0=mybir.AluOpType.mult)
                    lrl = mpool.tile([P, S * 8], fp16, tag="lrl")
                    nc.vector.tensor_tensor(out=lrl[:], in0=aex[:], in1=lrl0[:],
                                            op=mybir.AluOpType.max)
                    ex = mpool.tile([P, S * 8], fp16, tag="ex")
                    nc.scalar.activation(out=ex[:], in_=lrl[:],
                                         func=mybir.ActivationFunctionType.Exp)

                    msg = opool.tile([P, S * P], fp16, tag="msg")
                    ex_b = ex[:].rearrange("p (s h o) -> p s h o", h=8, o=1)
                    nc.vector.tensor_tensor(
                        out=msg[:].rearrange("p (s h c) -> p s h c", h=8, c=16),
                        in0=g3[:, :, 0:128].rearrange("p s (h c) -> p s h c", c=16),
                        in1=ex_b.to_broadcast([P, S, 8, 16]),
                        op=mybir.AluOpType.mult)

                    pd = epsum.tile([P, 128], f32, tag="epsum")
                    pde = epsum.tile([P, 8], f32, tag="epsden")
                    for s in range(S):
                        nc.tensor.matmul(out=pd[:, :],
                                         lhsT=oneh[:, s * P:(s + 1) * P],
                                         rhs=msg[:, s * P:(s + 1) * P],
                                         start=(s == 0), stop=(s == S - 1))
                    for s in range(S):
                        nc.tensor.matmul(out=pde[:, :],
                                         lhsT=oneh[:, s * P:(s + 1) * P],
                                         rhs=ex[:, s * 8:(s + 1) * 8],
                                         start=(s == 0), stop=(s == S - 1))

                    den = mpool.tile([P, 8], f32, tag="den")
                    nc.vector.tensor_scalar(out=den[:], in0=pde[:, :],
                                            scalar1=1e-30, scalar2=None,
                                            op0=mybir.AluOpType.add)
                    rec = mpool.tile([P, 8], f32, tag="rec")
                    nc.vector.reciprocal(out=rec[:], in_=den[:])
                    od = dpool.tile([P, 128], f32, tag="od")
                    rec_b = rec[:].rearrange("p (h o) -> p h o", o=1)
                    nc.vector.tensor_tensor(
                        out=od[:].rearrange("p (h c) -> p h c", c=16),
                        in0=pd[:, :].rearrange("p (h c) -> p h c", c=16),
                        in1=rec_b.to_broadcast([P, 8, 16]),
                        op=mybir.AluOpType.mult)
                    outs_d.append(od)

                osum = dpool.tile([P, 128], f32, tag="osum")
                nc.vector.tensor_tensor(out=osum[:], in0=outs_d[0][:],
                                        in1=outs_d[1][:], op=mybir.AluOpType.add)
                ofin = dpool.tile([P, 128], f32, tag="ofin")
                nc.vector.tensor_tensor(out=ofin[:], in0=osum[:], in1=bias_s[:],
                                        op=mybir.AluOpType.add)
                nc.sync.dma_start(out=p_out[b * P:(b + 1) * P, :], in_=ofin[:])

    nc.compile()
    ctx.close()

    shared = {"xT": xT, "wcat": wcat, "iota": iota, "sent": sent, "bias": bias}
    in_maps = [dict(shared, einfo=einfo[k], ldst=ldsta[k]) for k in range(n_cores)]
    _LAST["nc"] = nc
    _LAST["in_maps"] = in_maps
    _LAST["n_cores"] = n_cores
    res = run_bass_kernel_spmd(nc, in_maps, list(range(n_cores)))
    full = np.concatenate([res.results[k]["out"] for k in range(n_cores)], axis=0)
    return full[:n_nodes].astype(np.float32)


def kernel(x, ei, W_in, a_src_in, a_dst_in, b_in, W_out, a_src_out, a_dst_out,
           b_out):
    x = np.asarray(x, np.float32)
    ei = np.asarray(ei, np.int32)
    return _build_and_run(
        x, ei,
        np.asarray(W_in, np.float32), np.asarray(a_src_in, np.float32),
        np.asarray(a_dst_in, np.float32), np.asarray(b_in, np.float32),
        np.asarray(W_out, np.float32), np.asarray(a_src_out, np.float32),
        np.asarray(a_dst_out, np.float32), np.asarray(b_out, np.float32),
        n_nodes=x.shape[0], n_edges=ei.shape[1])

